# revision 1
# baseline (speedup 1.0000x reference)
"""Trainium2 Bass kernel for the NodeAttentionLayer (GAT-style) problem.

Math (per reference.py):
    h_t = t_input @ W_t; h_o = o_input @ W_o
    s_t = h_t @ a[:F];  s_o = h_o @ a[F:]
    e[i,j]   = leaky_relu(s_t[i] + s_o[j], 0.2)
    att      = softmax(where(adj>0, e, -9e15), axis=1)
    out      = elu(att @ h_o)

Key identity used on-device: with y = s_t[i] + s_o[j], c = (y > 0),
    exp(lrelu(y)) = c * u1[i] v1[j] + (1-c) * u2[i] v2[j]
where u1 = exp(s_t), v1 = exp(s_o), u2 = exp(0.2 s_t), v2 = exp(0.2 s_o).
So with M1 = adj * c and M2 = adj - M1 (both 0/1 masks):
    att-numer @ h_o = u1 * (v1*h_o_ext).T @ M1 + u2 * (v2*h_o_ext).T @ M2
(h_o_ext = [h_o | 1] supplies the softmax denominator as row F). Softmax and
the exp of the max-trick cancel in the ratio, and dividing numerator and
denominator by u2 leaves a single per-column factor r = exp(0.8 s_t).

Sharding: rows of t_input/adj (N_t) split across 8 cores; o_input replicated.
The kernel computes output TRANSPOSED ([F, rows]) per core; host transposes
back.  adj is fed per-core as adj[rows,:].T in bf16 (0/1 -> lossless).
"""

import contextlib
import ctypes
import sys
import tempfile
import types

import ml_dtypes
import numpy as np

import concourse.bass as bass
import concourse.mybir as mybir
import concourse.tile as tile
from concourse.vector_clock import ScopedClock

bf16 = ml_dtypes.bfloat16

# ---------------------------------------------------------------------------
# Environment shims
# ---------------------------------------------------------------------------

def _patch_tile_drain():
    """walrus in this container allows only one sync-wait per sync-engine
    instruction; split the TileContext epilogue drain's waits onto
    individual nops."""
    if getattr(tile.TileContext, "_drain_patch_installed", False):
        return

    def _drain_and_barrier(self, tick_clock, wait_clock):
        nop_inst = self.nc.sync.nop(nofuse=True)
        wait_clock.add_sem_waits(
            nop_inst.ins, ScopedClock({None: tick_clock.global_clock})
        )
        ow = list(nop_inst.ins.sync_info.on_wait) if nop_inst.ins.sync_info else []
        if len(ow) > 1:
            nop_inst.ins.sync_info.on_wait = ow[:1]
            for w in ow[1:]:
                extra = self.nc.sync.nop(nofuse=True)
                if extra.ins.sync_info is None:
                    extra.ins.sync_info = mybir.SyncInfo(on_wait=[w], on_update=[])
                else:
                    extra.ins.sync_info.on_wait = [w]
        self.nc.sync.drain()
        self.nc.all_engine_barrier()
        popped = self.nc._tile_sem_poison_stack.pop()
        assert popped is self._sem_poison
        self.nc.clear_and_free_semaphores(list(self.sems.allocated().values()))
        self.nc.all_engine_barrier()

    tile.TileContext._drain_and_barrier = _drain_and_barrier
    tile.TileContext._drain_patch_installed = True


def _install_ntff_hook():
    """Provide antenv.axon_hooks (absent in this image) so trace=True works."""
    if "antenv.axon_hooks" in sys.modules:
        return
    import antenv

    state = {"hook": None}
    mod = types.ModuleType("antenv.axon_hooks")
    mod.set_axon_ntff_profile_hook = lambda h: state.__setitem__("hook", h)
    mod.get_axon_ntff_profile_hook = lambda: state["hook"]
    sys.modules["antenv.axon_hooks"] = mod
    antenv.axon_hooks = mod

    try:
        lib = ctypes.CDLL("/opt/axon/libaxon_pjrt.so")
    except OSError:
        return
    if not hasattr(lib, "axon_start_nrt_profile"):
        return
    lib.axon_start_nrt_profile.argtypes = [
        ctypes.POINTER(ctypes.c_int64),
        ctypes.c_size_t,
    ]
    lib.axon_start_nrt_profile.restype = ctypes.c_int64
    lib.axon_stop_nrt_profile.argtypes = [ctypes.c_char_p]
    lib.axon_stop_nrt_profile.restype = ctypes.c_int64

    @contextlib.contextmanager
    def _ntff_hook(output_dir, device_ids):
        import jax

        jax.devices()
        if device_ids:
            ids = (ctypes.c_int64 * len(device_ids))(*device_ids)
            rc = lib.axon_start_nrt_profile(ids, len(device_ids))
        else:
            rc = lib.axon_start_nrt_profile(None, 0)
        if rc != 0:
            raise RuntimeError(f"axon_start_nrt_profile rc={rc}")
        try:
            yield
        finally:
            n = lib.axon_stop_nrt_profile(str(output_dir).encode())
            print(f"profile: {n} file(s) written to {output_dir}", file=sys.stderr)

    state["hook"] = _ntff_hook


_patch_tile_drain()
_install_ntff_hook()


def _split_multi_waits(nc):
    """walrus here accepts at most ONE sync-wait per instruction; hoist extra
    waits onto same-engine nops inserted immediately before."""
    import bass_rust

    k = 0
    for f in nc.m.functions:
        for blk in f.blocks:
            insts = blk.instructions
            out = []
            changed = False
            for inst in insts:
                si = inst.sync_info
                ow = list(si.on_wait) if si is not None else []
                if len(ow) > 1:
                    for w in ow[:-1]:
                        nop = bass_rust.InstNoOp(
                            name=f"waitsplit-{k}", engine=inst.engine
                        )
                        k += 1
                        nop.sync_info = mybir.SyncInfo(on_wait=[w], on_update=[])
                        out.append(nop)
                    si.on_wait = [ow[-1]]
                    changed = True
                out.append(inst)
            if changed:
                blk.instructions = out

# ---------------------------------------------------------------------------
# Problem constants (hardcoded per spec)
# ---------------------------------------------------------------------------
N_T, N_O, F_IN, F_OUT = 8192, 8192, 256, 64
N_CORES = 8
R = N_T // N_CORES            # rows (i) per core = 1024
NJ = N_O // 128               # j tiles of 128 = 64
KC = F_IN // 128              # contraction chunks for projections = 2
GROUP = 16                    # j-tiles per setup group
NG = NJ // GROUP              # setup groups = 4
ALPHA = 0.2
F32 = mybir.dt.float32
BF16 = mybir.dt.bfloat16
AF = mybir.ActivationFunctionType
OP = mybir.AluOpType


def _ap_bcast_partitions(ap, n):
    """AP view replicating a [1, ...] access pattern across n partitions."""
    return bass.AP(tensor=ap.tensor, offset=ap.offset, ap=[[0, n]] + list(ap.ap[1:]))


def _ap_repeat_free(ap, reps):
    """AP view of a [P, K] tile as [P, K, reps] (innermost step-0 repeat)."""
    return bass.AP(
        tensor=ap.tensor, offset=ap.offset, ap=list(ap.ap) + [[0, reps]]
    )


def build_kernel(split_waits=True):
    nc = bass.Bass("TRN2")

    t_T = nc.dram_tensor("t_T", [F_IN, R], F32, kind="ExternalInput")
    o_T = nc.dram_tensor("o_T", [F_IN, N_O], F32, kind="ExternalInput")
    w_t = nc.dram_tensor("w_t", [F_IN, F_OUT], F32, kind="ExternalInput")
    w_o = nc.dram_tensor("w_o", [F_IN, F_OUT], F32, kind="ExternalInput")
    a_vec = nc.dram_tensor("a_vec", [2 * F_OUT, 1], F32, kind="ExternalInput")
    adjT = nc.dram_tensor("adjT", [N_O, R], BF16, kind="ExternalInput")
    out = nc.dram_tensor("out", [F_OUT, R], F32, kind="ExternalOutput")

    with tile.TileContext(nc) as tc, contextlib.ExitStack() as ctx:
        singles = ctx.enter_context(tc.tile_pool(name="singles", bufs=1))
        stage = ctx.enter_context(tc.tile_pool(name="stage", bufs=2))
        adj_pool = ctx.enter_context(tc.tile_pool(name="adj", bufs=6))
        c_pool = ctx.enter_context(tc.tile_pool(name="cmask", bufs=4))
        m1_pool = ctx.enter_context(tc.tile_pool(name="m1", bufs=4))
        m2_pool = ctx.enter_context(tc.tile_pool(name="m2", bufs=4))
        acc_psum = ctx.enter_context(tc.tile_pool(name="acc", bufs=1, space="PSUM"))
        misc_psum = ctx.enter_context(tc.tile_pool(name="mpsum", bufs=2, space="PSUM"))

        # ------------------------------------------------------------------
        # Setup: weights + t-side scalars
        # ------------------------------------------------------------------
        wt_sb = singles.tile([128, KC, F_OUT], F32)
        wo_sb = singles.tile([128, KC, F_OUT], F32)
        for c in range(KC):
            nc.sync.dma_start(out=wt_sb[:, c, :], in_=w_t[c * 128:(c + 1) * 128, :])
            nc.sync.dma_start(out=wo_sb[:, c, :], in_=w_o[c * 128:(c + 1) * 128, :])
        a_t_sb = singles.tile([F_OUT, 1], F32)
        nc.sync.dma_start(out=a_t_sb[:, :], in_=a_vec[0:F_OUT, :])
        # a_o broadcast to [128, F_OUT]
        a_o_b = singles.tile([128, F_OUT], F32)
        nc.sync.dma_start(
            out=a_o_b[:, :],
            in_=bass.AP(tensor=a_vec, offset=F_OUT, ap=[[0, 128], [1, F_OUT]]),
        )
        ones_row = singles.tile([1, F_OUT + 1], F32)
        nc.vector.memset(ones_row[:, :], 1.0)

        t_T_sb = singles.tile([128, KC, R], F32)
        for c in range(KC):
            nc.sync.dma_start(out=t_T_sb[:, c, :], in_=t_T[c * 128:(c + 1) * 128, :])

        # h_tT [F_OUT, R] = W_t.T @ t_blk.T   (psum, 2 k-chunks x 2 n-chunks)
        ht_sb = singles.tile([F_OUT, R], F32)
        for n in range(R // 512):
            ht_ps = misc_psum.tile([F_OUT, 512], F32, tag="mps")
            for c in range(KC):
                nc.tensor.matmul(
                    ht_ps[:, :],
                    wt_sb[:, c, :],
                    t_T_sb[:, c, n * 512:(n + 1) * 512],
                    start=(c == 0),
                    stop=(c == KC - 1),
                )
            nc.vector.tensor_copy(ht_sb[:, n * 512:(n + 1) * 512], ht_ps[:, :])

        # s_t row [1, R]; r = exp(0.8 s_t) row; bf16 s_t row
        st_row = singles.tile([1, R], F32)
        r_row = singles.tile([1, R], F32)
        st_row_b = singles.tile([1, R], BF16)
        for n in range(R // 512):
            st_ps = misc_psum.tile([1, 512], F32, tag="mps")
            nc.tensor.matmul(
                st_ps[:, :],
                a_t_sb[:, :],
                ht_sb[:, n * 512:(n + 1) * 512],
                start=True,
                stop=True,
            )
            nc.vector.tensor_copy(st_row[:, n * 512:(n + 1) * 512], st_ps[:, :])
            nc.scalar.activation(
                r_row[:, n * 512:(n + 1) * 512], st_ps[:, :], AF.Exp, scale=0.8
            )
        nc.vector.tensor_copy(st_row_b[:, :], st_row[:, :])

        # s_t broadcast to all partitions [128, R] bf16 (via DRAM bounce --
        # partition-step-0 APs are only legal on DRAM sources)
        st_dram = nc.dram_tensor("st_bounce", [1, R], BF16, kind="Internal")
        nc.sync.dma_start(out=st_dram[:, :], in_=st_row_b[0:1, :])
        st_bcast = singles.tile([128, R], BF16)
        nc.sync.dma_start(
            out=st_bcast[:, :],
            in_=bass.AP(tensor=st_dram, offset=0, ap=[[0, 128], [1, R]]),
        )

        # o_input.T resident for projections
        o_T_sb = singles.tile([128, KC, N_O], F32)
        for c in range(KC):
            for h in range(2):
                nc.sync.dma_start(
                    out=o_T_sb[:, c, h * 4096:(h + 1) * 4096],
                    in_=o_T[c * 128:(c + 1) * 128, h * 4096:(h + 1) * 4096],
                )

        # ------------------------------------------------------------------
        # Per-group o-side setup: h_o, s_o, v1/v2, W1ext/W2ext (bf16)
        # ------------------------------------------------------------------
        w1_tiles, w2_tiles, nso_tiles = [], [], []
        for g in range(NG):
            ho_stage = stage.tile([128, GROUP, F_OUT], F32, tag="ho_stage")
            for u in range(0, GROUP, 8):
                ho_ps = misc_psum.tile([128, 8, F_OUT], F32, tag="mps")
                for s in range(8):
                    j0 = (g * GROUP + u + s) * 128
                    for c in range(KC):
                        nc.tensor.matmul(
                            ho_ps[:, s, :],
                            o_T_sb[:, c, j0:j0 + 128],
                            wo_sb[:, c, :],
                            start=(c == 0),
                            stop=(c == KC - 1),
                        )
                nc.vector.tensor_copy(ho_stage[:, u:u + 8, :], ho_ps[:, :, :])

            # s_o[j] = sum_f h_o[j,f] a_o[f]
            so_g = stage.tile([128, GROUP], F32, tag="so")
            prod = stage.tile([128, GROUP, F_OUT], F32, tag="so_prod")
            nc.vector.tensor_tensor(
                prod[:, :, :],
                ho_stage[:, :, :],
                bass.AP(
                    tensor=a_o_b[:, :].tensor,
                    offset=a_o_b[:, :].offset,
                    ap=[a_o_b[:, :].ap[0], [0, GROUP], [1, F_OUT]],
                ),
                OP.mult,
            )
            nc.vector.tensor_reduce(
                so_g[:, :], prod[:, :, :], mybir.AxisListType.X, OP.add
            )
            nso_g = singles.tile([128, GROUP], F32, tag=f"nso{g}")
            nc.vector.tensor_scalar_mul(nso_g[:, :], so_g[:, :], -1.0)
            v1_g = stage.tile([128, GROUP], F32, tag="v1")
            v2_g = stage.tile([128, GROUP], F32, tag="v2")
            nc.scalar.activation(v1_g[:, :], so_g[:, :], AF.Exp)
            nc.scalar.activation(v2_g[:, :], so_g[:, :], AF.Exp, scale=ALPHA)

            w1_g = singles.tile([128, GROUP, F_OUT + 1], BF16, tag=f"w1{g}")
            w2_g = singles.tile([128, GROUP, F_OUT + 1], BF16, tag=f"w2{g}")
            # scaled h_o columns (on gpsimd to offload DVE)
            nc.gpsimd.tensor_tensor(
                w1_g[:, :, 0:F_OUT],
                ho_stage[:, :, :],
                _ap_repeat_free(v1_g[:, :], F_OUT),
                OP.mult,
            )
            nc.gpsimd.tensor_tensor(
                w2_g[:, :, 0:F_OUT],
                ho_stage[:, :, :],
                _ap_repeat_free(v2_g[:, :], F_OUT),
                OP.mult,
            )
            # ones columns carry v1/v2 for the softmax denominator
            nc.vector.tensor_copy(w1_g[:, :, F_OUT], v1_g[:, :])
            nc.vector.tensor_copy(w2_g[:, :, F_OUT], v2_g[:, :])
            w1_tiles.append(w1_g)
            w2_tiles.append(w2_g)
            nso_tiles.append(nso_g)

        # ------------------------------------------------------------------
        # Main loop over j tiles: masks + accumulating matmuls
        # ------------------------------------------------------------------
        NI2 = R // 512  # 2 matmul chunks over i
        t1_acc = [
            acc_psum.tile([F_OUT + 1, 512], F32, tag=f"t1_{n}", name=f"t1_acc{n}")
            for n in range(NI2)
        ]
        t2_acc = [
            acc_psum.tile([F_OUT + 1, 512], F32, tag=f"t2_{n}", name=f"t2_acc{n}")
            for n in range(NI2)
        ]

        for t in range(NJ):
            g, u = divmod(t, GROUP)
            adj_t = adj_pool.tile([128, R], BF16)
            nc.sync.dma_start(out=adj_t[:, :], in_=adjT[t * 128:(t + 1) * 128, :])

            c_t = c_pool.tile([128, R], BF16)
            nc.vector.tensor_scalar(
                c_t[:, :], st_bcast[:, :], nso_tiles[g][:, u:u + 1], None, OP.is_gt
            )
            m1_t = m1_pool.tile([128, R], BF16)
            nc.vector.tensor_tensor(m1_t[:, :], c_t[:, :], adj_t[:, :], OP.mult)
            m2_t = m2_pool.tile([128, R], BF16)
            nc.gpsimd.tensor_tensor(m2_t[:, :], adj_t[:, :], m1_t[:, :], OP.subtract)

            for n in range(NI2):
                nc.tensor.matmul(
                    t1_acc[n][:, :],
                    w1_tiles[g][:, u, :],
                    m1_t[:, n * 512:(n + 1) * 512],
                    start=(t == 0),
                    stop=(t == NJ - 1),
                )
            for n in range(NI2):
                nc.tensor.matmul(
                    t2_acc[n][:, :],
                    w2_tiles[g][:, u, :],
                    m2_t[:, n * 512:(n + 1) * 512],
                    start=(t == 0),
                    stop=(t == NJ - 1),
                )

        # ------------------------------------------------------------------
        # Combine: H = r*T1 + T2 ; out = elu(H[:F] / H[F])
        # ------------------------------------------------------------------
        h_sb = singles.tile([F_OUT + 1, R], F32)
        for n in range(NI2):
            sl = slice(n * 512, (n + 1) * 512)
            rb_ps = misc_psum.tile([F_OUT + 1, 512], F32, tag="mps")
            nc.tensor.matmul(
                rb_ps[:, :], ones_row[:, :], r_row[:, sl], start=True, stop=True
            )
            rb_sb = stage.tile([F_OUT + 1, 512], F32, tag="rb_sb")
            nc.vector.tensor_copy(rb_sb[:, :], rb_ps[:, :])
            nc.vector.tensor_tensor(h_sb[:, sl], rb_sb[:, :], t1_acc[n][:, :], OP.mult)
            nc.vector.tensor_tensor(h_sb[:, sl], h_sb[:, sl], t2_acc[n][:, :], OP.add)

        zr_row = singles.tile([1, R], F32)
        nc.vector.reciprocal(zr_row[:, :], h_sb[F_OUT:F_OUT + 1, :])

        ot_sb = singles.tile([F_OUT, R], F32)
        for n in range(NI2):
            sl = slice(n * 512, (n + 1) * 512)
            zb_ps = misc_psum.tile([F_OUT, 512], F32, tag="mps")
            nc.tensor.matmul(
                zb_ps[:, :], ones_row[:, 0:F_OUT], zr_row[:, sl], start=True, stop=True
            )
            nc.vector.tensor_tensor(ot_sb[:, sl], h_sb[0:F_OUT, sl], zb_ps[:, :], OP.mult)

        # elu(x) = max(x,0) - 1 + exp(min(x,0))
        mn_sb = singles.tile([F_OUT, R], F32)
        ex_sb = singles.tile([F_OUT, R], F32)
        nc.vector.tensor_scalar(mn_sb[:, :], ot_sb[:, :], 0.0, None, OP.min)
        nc.scalar.activation(ex_sb[:, :], mn_sb[:, :], AF.Exp)
        nc.vector.tensor_scalar(ot_sb[:, :], ot_sb[:, :], 0.0, -1.0, OP.max, OP.add)
        nc.vector.tensor_tensor(ot_sb[:, :], ot_sb[:, :], ex_sb[:, :], OP.add)
        nc.sync.dma_start(out=out[:, :], in_=ot_sb[:, :])

    if split_waits:
        _split_multi_waits(nc)
    return nc


_CACHED = {}


def _get_compiled():
    if "nc" not in _CACHED:
        _CACHED["nc"] = build_kernel()
    return _CACHED["nc"]


def kernel(t_input, o_input, W_t, W_o, a, adj, _trace=False):
    from concourse.bass_utils import run_bass_kernel_spmd

    t_input = np.asarray(t_input, dtype=np.float32)
    o_input = np.asarray(o_input, dtype=np.float32)
    W_t = np.asarray(W_t, dtype=np.float32)
    W_o = np.asarray(W_o, dtype=np.float32)
    a = np.asarray(a, dtype=np.float32)
    adj = np.asarray(adj)

    o_T = np.ascontiguousarray(o_input.T)
    adj_b = adj.astype(bf16)

    in_maps = []
    for m in range(N_CORES):
        rows = slice(m * R, (m + 1) * R)
        in_maps.append(
            {
                "t_T": np.ascontiguousarray(t_input[rows, :].T),
                "o_T": o_T,
                "w_t": W_t,
                "w_o": W_o,
                "a_vec": a,
                "adjT": np.ascontiguousarray(adj_b[rows, :].T),
            }
        )

    nc = _get_compiled()
    res = run_bass_kernel_spmd(
        nc, in_maps, core_ids=list(range(N_CORES)), trace=_trace
    )
    out = np.empty((N_T, F_OUT), dtype=np.float32)
    for m in range(N_CORES):
        out[m * R:(m + 1) * R, :] = res.results[m]["out"].T
    if _trace:
        kernel.last_exec_time_ns = res.exec_time_ns
        kernel.last_results = res
    return out



# revision 2
# speedup vs baseline: 2.3228x; 2.3228x over previous
"""Trainium2 Bass kernel for the NodeAttentionLayer (GAT-style) problem.

Math (per reference.py):
    h_t = t_input @ W_t; h_o = o_input @ W_o
    s_t = h_t @ a[:F];  s_o = h_o @ a[F:]
    e[i,j]   = leaky_relu(s_t[i] + s_o[j], 0.2)
    att      = softmax(where(adj>0, e, -9e15), axis=1)
    out      = elu(att @ h_o)

Single-mask identity used on-device:
    exp(lrelu(y)) = exp(0.2 y) * max(exp(0.8 y), 1),   y = s_t[i] + s_o[j]
The i-side factor exp(0.2 s_t) cancels in the softmax ratio, so with
    q[j,i] = max(exp(0.8 s_t_i) * exp(s_o_j), exp(0.2 s_o_j))
           = exp(0.2 s_o_j) * max(exp(0.8 y), 1)
the (unnormalized, u2-cancelled) attention is m = q * adj and
    out[:,i] = elu( (W_ext^T m)[0:F,i] / (W_ext^T m)[F,i] ),  W_ext = [h_o | 1].
q is ONE dual-op tensor_scalar (mult,max with two per-partition scalar
columns) and m ONE tensor_tensor mult per j-tile — both on DVE in 2x/4x
modes.  One matmul pair per tile accumulates in PSUM over all 64 tiles
with a fixed 65-column stationary per tile.  GpSimd is untouched (its
SBUF port contention slows DVE ~4x).

h_t is never materialized: s_t = t_input @ (W_t a_t) is a matvec, and
s_o rides as a 65th projection column ([W_o | W_o a_o] moving operand).

Sharding: rows of t_input/adj (N_t) split across 8 cores; o replicated.
Kernel computes output TRANSPOSED ([F, rows]) per core; host transposes.
adj fed per-core as adj[rows,:].T in bf16 (0/1 -> lossless); t/o in bf16.
"""

import contextlib
import ctypes
import sys
import tempfile
import types

import ml_dtypes
import numpy as np

import concourse.bass as bass
import concourse.mybir as mybir
import concourse.tile as tile
from concourse.vector_clock import ScopedClock

bf16 = ml_dtypes.bfloat16

# ---------------------------------------------------------------------------
# Environment shims
# ---------------------------------------------------------------------------

def _patch_tile_drain():
    """walrus in this container allows only one sync-wait per sync-engine
    instruction; split the TileContext epilogue drain's waits onto
    individual nops."""
    if getattr(tile.TileContext, "_drain_patch_installed", False):
        return

    def _drain_and_barrier(self, tick_clock, wait_clock):
        nop_inst = self.nc.sync.nop(nofuse=True)
        wait_clock.add_sem_waits(
            nop_inst.ins, ScopedClock({None: tick_clock.global_clock})
        )
        ow = list(nop_inst.ins.sync_info.on_wait) if nop_inst.ins.sync_info else []
        if len(ow) > 1:
            nop_inst.ins.sync_info.on_wait = ow[:1]
            for w in ow[1:]:
                extra = self.nc.sync.nop(nofuse=True)
                if extra.ins.sync_info is None:
                    extra.ins.sync_info = mybir.SyncInfo(on_wait=[w], on_update=[])
                else:
                    extra.ins.sync_info.on_wait = [w]
        self.nc.sync.drain()
        self.nc.all_engine_barrier()
        popped = self.nc._tile_sem_poison_stack.pop()
        assert popped is self._sem_poison
        self.nc.clear_and_free_semaphores(list(self.sems.allocated().values()))
        self.nc.all_engine_barrier()

    tile.TileContext._drain_and_barrier = _drain_and_barrier
    tile.TileContext._drain_patch_installed = True


def _install_ntff_hook():
    """Provide antenv.axon_hooks (absent in this image) so trace=True works."""
    if "antenv.axon_hooks" in sys.modules:
        return
    import antenv

    state = {"hook": None}
    mod = types.ModuleType("antenv.axon_hooks")
    mod.set_axon_ntff_profile_hook = lambda h: state.__setitem__("hook", h)
    mod.get_axon_ntff_profile_hook = lambda: state["hook"]
    sys.modules["antenv.axon_hooks"] = mod
    antenv.axon_hooks = mod

    try:
        lib = ctypes.CDLL("/opt/axon/libaxon_pjrt.so")
    except OSError:
        return
    if not hasattr(lib, "axon_start_nrt_profile"):
        return
    lib.axon_start_nrt_profile.argtypes = [
        ctypes.POINTER(ctypes.c_int64),
        ctypes.c_size_t,
    ]
    lib.axon_start_nrt_profile.restype = ctypes.c_int64
    lib.axon_stop_nrt_profile.argtypes = [ctypes.c_char_p]
    lib.axon_stop_nrt_profile.restype = ctypes.c_int64

    @contextlib.contextmanager
    def _ntff_hook(output_dir, device_ids):
        import jax

        jax.devices()
        if device_ids:
            ids = (ctypes.c_int64 * len(device_ids))(*device_ids)
            rc = lib.axon_start_nrt_profile(ids, len(device_ids))
        else:
            rc = lib.axon_start_nrt_profile(None, 0)
        if rc != 0:
            raise RuntimeError(f"axon_start_nrt_profile rc={rc}")
        try:
            yield
        finally:
            n = lib.axon_stop_nrt_profile(str(output_dir).encode())
            print(f"profile: {n} file(s) written to {output_dir}", file=sys.stderr)

    state["hook"] = _ntff_hook


_patch_tile_drain()
_install_ntff_hook()


def _split_multi_waits(nc):
    """walrus here accepts at most ONE sync-wait per instruction; hoist extra
    waits onto same-engine nops inserted immediately before."""
    import bass_rust

    k = 0
    for f in nc.m.functions:
        for blk in f.blocks:
            insts = blk.instructions
            out = []
            changed = False
            for inst in insts:
                si = inst.sync_info
                ow = list(si.on_wait) if si is not None else []
                if len(ow) > 1:
                    for w in ow[:-1]:
                        nop = bass_rust.InstNoOp(
                            name=f"waitsplit-{k}", engine=inst.engine
                        )
                        k += 1
                        nop.sync_info = mybir.SyncInfo(on_wait=[w], on_update=[])
                        out.append(nop)
                    si.on_wait = [ow[-1]]
                    changed = True
                out.append(inst)
            if changed:
                blk.instructions = out

# ---------------------------------------------------------------------------
# Problem constants (hardcoded per spec)
# ---------------------------------------------------------------------------
N_T, N_O, F_IN, F_OUT = 8192, 8192, 256, 64
N_CORES = 8
R = N_T // N_CORES            # rows (i) per core = 1024
NJ = N_O // 128               # j tiles of 128 = 64
KC = F_IN // 128              # contraction chunks for projections = 2
GROUP = 16                    # j-tiles per setup group
NG = NJ // GROUP              # setup groups = 4
FX = F_OUT + 1                # 65 (h_o columns + ones/denominator column)
F32 = mybir.dt.float32
BF16 = mybir.dt.bfloat16
AF = mybir.ActivationFunctionType
OP = mybir.AluOpType


def _ap(tensor, offset, ap):
    return bass.AP(tensor=tensor, offset=offset, ap=ap)


def build_kernel(split_waits=True):
    nc = bass.Bass("TRN2")

    t_T = nc.dram_tensor("t_T", [F_IN, R], BF16, kind="ExternalInput")
    o_T = nc.dram_tensor("o_T", [F_IN, N_O], BF16, kind="ExternalInput")
    w_t = nc.dram_tensor("w_t", [F_IN, F_OUT], F32, kind="ExternalInput")
    w_o = nc.dram_tensor("w_o", [F_IN, F_OUT], F32, kind="ExternalInput")
    a_vec = nc.dram_tensor("a_vec", [2 * F_OUT, 1], F32, kind="ExternalInput")
    adjT = nc.dram_tensor("adjT", [N_O, R], BF16, kind="ExternalInput")
    out = nc.dram_tensor("out", [F_OUT, R], F32, kind="ExternalOutput")
    u8_dram = nc.dram_tensor("u8_bounce", [1, R], BF16, kind="Internal")
    den_dram = nc.dram_tensor("den_bounce", [1, R], F32, kind="Internal")
    zr_dram = nc.dram_tensor("zr_bounce", [1, R], F32, kind="Internal")

    with tile.TileContext(nc) as tc, contextlib.ExitStack() as ctx:
        singles = ctx.enter_context(tc.tile_pool(name="singles", bufs=1))
        stage = ctx.enter_context(tc.tile_pool(name="stage", bufs=2))
        adj_pool = ctx.enter_context(tc.tile_pool(name="adj", bufs=6))
        q_pool = ctx.enter_context(tc.tile_pool(name="q", bufs=3))
        m_pool = ctx.enter_context(tc.tile_pool(name="m", bufs=3))
        acc_psum = ctx.enter_context(tc.tile_pool(name="acc", bufs=1, space="PSUM"))
        misc_psum = ctx.enter_context(tc.tile_pool(name="mpsum", bufs=2, space="PSUM"))

        # ------------------------------------------------------------------
        # Weights + a-vector broadcasts
        # ------------------------------------------------------------------
        wt_sb = singles.tile([128, KC, F_OUT], F32)
        wo_sb = singles.tile([128, KC, F_OUT], F32)
        for c in range(KC):
            nc.sync.dma_start(out=wt_sb[:, c, :], in_=w_t[c * 128:(c + 1) * 128, :])
            nc.sync.dma_start(out=wo_sb[:, c, :], in_=w_o[c * 128:(c + 1) * 128, :])
        at_b = singles.tile([128, F_OUT], F32)
        ao_b = singles.tile([128, F_OUT], F32)
        nc.sync.dma_start(out=at_b[:, :], in_=_ap(a_vec, 0, [[0, 128], [1, F_OUT]]))
        nc.sync.dma_start(
            out=ao_b[:, :], in_=_ap(a_vec, F_OUT, [[0, 128], [1, F_OUT]])
        )

        # wta = W_t @ a_t (bf16 stationary), woa = W_o @ a_o (f32, 65th col)
        prod_t = stage.tile([128, KC, F_OUT], F32, tag="prod")
        nc.vector.tensor_tensor(
            prod_t[:, :, :],
            wt_sb[:, :, :],
            _ap(at_b[:, :].tensor, at_b[:, :].offset,
                [at_b[:, :].ap[0], [0, KC], [1, F_OUT]]),
            OP.mult,
        )
        red_t = stage.tile([128, KC], F32, tag="red")
        nc.vector.tensor_reduce(red_t[:, :], prod_t[:, :, :], mybir.AxisListType.X, OP.add)
        wta = singles.tile([128, KC], BF16)
        nc.vector.tensor_copy(wta[:, :], red_t[:, :])

        prod_o = stage.tile([128, KC, F_OUT], F32, tag="prod")
        nc.vector.tensor_tensor(
            prod_o[:, :, :],
            wo_sb[:, :, :],
            _ap(ao_b[:, :].tensor, ao_b[:, :].offset,
                [ao_b[:, :].ap[0], [0, KC], [1, F_OUT]]),
            OP.mult,
        )
        woa_f = stage.tile([128, KC], F32, tag="red")
        nc.vector.tensor_reduce(woa_f[:, :], prod_o[:, :, :], mybir.AxisListType.X, OP.add)

        # moving operand for the o-projection: [W_o | W_o a_o] in bf16
        woe = singles.tile([128, KC, FX], BF16)
        nc.vector.tensor_copy(woe[:, :, 0:F_OUT], wo_sb[:, :, :])
        nc.vector.tensor_copy(woe[:, :, F_OUT], woa_f[:, :])

        # ------------------------------------------------------------------
        # t side: s_t = t @ wta -> u8 = exp(0.8 s_t), broadcast to 128 parts
        # ------------------------------------------------------------------
        t_sb = singles.tile([128, KC, R], BF16)
        for c in range(KC):
            nc.sync.dma_start(out=t_sb[:, c, :], in_=t_T[c * 128:(c + 1) * 128, :])
        u8_row = singles.tile([1, R], BF16)
        for n in range(R // 512):
            st_ps = misc_psum.tile([1, 512], F32, tag="stps")
            for c in range(KC):
                nc.tensor.matmul(
                    st_ps[:, :],
                    wta[:, c:c + 1],
                    t_sb[:, c, n * 512:(n + 1) * 512],
                    start=(c == 0),
                    stop=(c == KC - 1),
                )
            nc.scalar.activation(
                u8_row[:, n * 512:(n + 1) * 512], st_ps[:, :], AF.Exp, scale=0.8
            )
        nc.sync.dma_start(out=u8_dram[:, :], in_=u8_row[0:1, :])
        u8_b = singles.tile([128, R], BF16)
        nc.sync.dma_start(out=u8_b[:, :], in_=_ap(u8_dram, 0, [[0, 128], [1, R]]))

        # ------------------------------------------------------------------
        # o side per group: h_o_ext = [h_o | s_o] -> hoe=[h_o|1], v1, v2
        # ------------------------------------------------------------------
        hoe, v1g, v2g = [], [], []
        for g in range(NG):
            o_sb = stage.tile([128, KC, GROUP * 128], BF16, tag="osb")
            for c in range(KC):
                nc.sync.dma_start(
                    out=o_sb[:, c, :],
                    in_=o_T[c * 128:(c + 1) * 128, g * GROUP * 128:(g + 1) * GROUP * 128],
                )
            hoe_g = singles.tile([128, GROUP, FX], BF16, tag=f"hoe{g}")
            v1_g = singles.tile([128, GROUP], F32, tag=f"v1{g}")
            v2_g = singles.tile([128, GROUP], F32, tag=f"v2{g}")
            for b in range(GROUP // 4):
                ho_ps = misc_psum.tile([128, 4, FX], F32, tag="hops")
                for s in range(4):
                    j0 = (b * 4 + s) * 128
                    for c in range(KC):
                        nc.tensor.matmul(
                            ho_ps[:, s, :],
                            o_sb[:, c, j0:j0 + 128],
                            woe[:, c, :],
                            start=(c == 0),
                            stop=(c == KC - 1),
                        )
                sl = slice(b * 4, b * 4 + 4)
                nc.scalar.activation(v1_g[:, sl], ho_ps[:, :, F_OUT], AF.Exp)
                nc.scalar.activation(
                    v2_g[:, sl], ho_ps[:, :, F_OUT], AF.Exp, scale=0.2
                )
                nc.vector.tensor_copy(hoe_g[:, sl, 0:F_OUT], ho_ps[:, :, 0:F_OUT])
            nc.vector.memset(hoe_g[:, :, F_OUT], 1.0)
            hoe.append(hoe_g)
            v1g.append(v1_g)
            v2g.append(v2_g)

        # ------------------------------------------------------------------
        # Main loop over j tiles: q = max(u8*v1, v2); m = q*adj; accumulate
        # ------------------------------------------------------------------
        acc = [
            acc_psum.tile([FX, 512], F32, tag=f"acc{n}", name=f"acc{n}")
            for n in range(2)
        ]
        for t in range(NJ):
            g, u = divmod(t, GROUP)
            adj_t = adj_pool.tile([128, R], BF16)
            nc.sync.dma_start(out=adj_t[:, :], in_=adjT[t * 128:(t + 1) * 128, :])
            q_t = q_pool.tile([128, R], BF16)
            nc.vector.tensor_scalar(
                q_t[:, :], u8_b[:, :], v1g[g][:, u:u + 1], v2g[g][:, u:u + 1],
                OP.mult, OP.max,
            )
            m_t = m_pool.tile([128, R], BF16)
            nc.vector.tensor_tensor(m_t[:, :], q_t[:, :], adj_t[:, :], OP.mult)
            for n in range(2):
                nc.tensor.matmul(
                    acc[n][:, :],
                    hoe[g][:, u, :],
                    m_t[:, n * 512:(n + 1) * 512],
                    start=(t == 0),
                    stop=(t == NJ - 1),
                )

        # ------------------------------------------------------------------
        # Tail: out = elu(T[0:F]/T[F])
        # ------------------------------------------------------------------
        h_sb = singles.tile([FX, R], F32)
        for n in range(2):
            nc.vector.tensor_copy(h_sb[:, n * 512:(n + 1) * 512], acc[n][:, :])
        # reciprocal of the denominator row via [128, R/128] reshape bounce
        nc.sync.dma_start(out=den_dram[:, :], in_=h_sb[F_OUT:FX, :])
        RP = R // 128
        d128 = singles.tile([128, RP], F32)
        nc.sync.dma_start(out=d128[:, :], in_=_ap(den_dram, 0, [[RP, 128], [1, RP]]))
        zr128 = singles.tile([128, RP], F32)
        nc.vector.reciprocal(zr128[:, :], d128[:, :])
        nc.sync.dma_start(out=_ap(zr_dram, 0, [[RP, 128], [1, RP]]), in_=zr128[:, :])
        zb = singles.tile([F_OUT, R], F32)
        nc.sync.dma_start(out=zb[:, :], in_=_ap(zr_dram, 0, [[0, F_OUT], [1, R]]))
        ot = singles.tile([F_OUT, R], F32)
        nc.vector.tensor_tensor(ot[:, :], h_sb[0:F_OUT, :], zb[:, :], OP.mult)
        # elu(x) = max(x,0) - 1 + exp(min(x,0))
        mn_sb = singles.tile([F_OUT, R], F32)
        ex_sb = singles.tile([F_OUT, R], F32)
        nc.vector.tensor_scalar(mn_sb[:, :], ot[:, :], 0.0, None, OP.min)
        nc.scalar.activation(ex_sb[:, :], mn_sb[:, :], AF.Exp)
        nc.vector.tensor_scalar(ot[:, :], ot[:, :], 0.0, -1.0, OP.max, OP.add)
        nc.vector.tensor_tensor(ot[:, :], ot[:, :], ex_sb[:, :], OP.add)
        nc.sync.dma_start(out=out[:, :], in_=ot[:, :])

    if split_waits:
        _split_multi_waits(nc)
    return nc


_CACHED = {}


def _get_compiled():
    if "nc" not in _CACHED:
        _CACHED["nc"] = build_kernel()
    return _CACHED["nc"]


def kernel(t_input, o_input, W_t, W_o, a, adj, _trace=False):
    from concourse.bass_utils import run_bass_kernel_spmd

    t_input = np.asarray(t_input, dtype=np.float32)
    o_input = np.asarray(o_input, dtype=np.float32)
    W_t = np.asarray(W_t, dtype=np.float32)
    W_o = np.asarray(W_o, dtype=np.float32)
    a = np.asarray(a, dtype=np.float32)
    adj = np.asarray(adj)

    o_T = np.ascontiguousarray(o_input.T).astype(bf16)
    adj_b = adj.astype(bf16)

    in_maps = []
    for m in range(N_CORES):
        rows = slice(m * R, (m + 1) * R)
        in_maps.append(
            {
                "t_T": np.ascontiguousarray(t_input[rows, :].T).astype(bf16),
                "o_T": o_T,
                "w_t": W_t,
                "w_o": W_o,
                "a_vec": a,
                "adjT": np.ascontiguousarray(adj_b[rows, :].T),
            }
        )

    nc = _get_compiled()
    res = run_bass_kernel_spmd(
        nc, in_maps, core_ids=list(range(N_CORES)), trace=_trace
    )
    out = np.empty((N_T, F_OUT), dtype=np.float32)
    for m in range(N_CORES):
        out[m * R:(m + 1) * R, :] = res.results[m]["out"].T
    if _trace:
        kernel.last_exec_time_ns = res.exec_time_ns
        kernel.last_results = res
    return out
